# revision 1
# baseline (speedup 1.0000x reference)
"""MixtureAttention (MoE attention routing) Trainium2 kernel.

Strategy: expert-parallel over 8 NeuronCores (one expert per core).
Each core computes its expert's full attention output for all tokens,
multiplies by the per-token router weight (0 for tokens that did not
select this expert in their top-2), and the host sums the 8 per-core
outputs — the top-k combine becomes a plain sum because the router
weight is zero for non-selected experts.

Router math on device (per token): top-2 of 8 logits == (m1, m2) the
two largest logits; softmax over them gives w1 = sigmoid(m1-m2),
w2 = 1-w1; this core's weight is w1/w2/0 by comparing its own logit
against m1/m2 (exact float equality; ties are measure-zero).

Layout: activations are kept transposed ("T-layout", feature on
partitions, token on free dim) so every matmul contracts along
partitions; q/k/v are pre-transposed on the host.  Per (batch,
512-token chunk):
  Q = wq^T qT (+bq, *hd^-0.5) -> per head:
  S^T[mk,nq] = K_h^T(lhsT) @ Q_h -> exp (ACT runs ONLY Exp — any other
  ACT function would thrash the activation table) -> AV with a ones
  column appended to V (65th column) so the softmax denominator lands
  in psum row 64 -> reciprocal (DVE) + PE-ones broadcast -> normalize
  -> O-proj (+bo, *router weight) -> out[b, d, token].

All big matmuls run in float32r (TF32-like, ~1e-4 rel err, 4x faster
than fp32 on the PE).  Everything else is fp32.
"""

import numpy as np

B, N, D, E, H = 2, 2048, 1024, 8, 16
MK = 512            # keys/values chunk per expert (M // E)
HD = D // H         # 64
P = 128
KO = D // P         # 8
NQC = 512           # token chunk (matmul free dim)
NCH = N // NQC      # 4
SCALE = HD ** -0.5
CORES = 8

_NC = None
import os
ABL = set(os.environ.get("KABL", "").split(","))
KREP = int(os.environ.get("KREP", "1"))


def _build_nc():
    import concourse.bacc as bacc
    import concourse.mybir as mybir
    from concourse.tile import TileContext
    from concourse.masks import make_identity

    f32 = mybir.dt.float32
    f32r = mybir.dt.float32r
    Af = mybir.ActivationFunctionType
    Op = mybir.AluOpType

    nc = bacc.Bacc("TRN2", target_bir_lowering=False)

    qT_d = nc.declare_dram_parameter("qT", [B, D, N], f32r, isOutput=False)
    kT_d = nc.declare_dram_parameter("kT", [B, D, MK], f32r, isOutput=False)
    vT_d = nc.declare_dram_parameter("vT", [B, D, MK], f32r, isOutput=False)
    wq_d = nc.declare_dram_parameter("wq", [D, D], f32r, isOutput=False)
    wk_d = nc.declare_dram_parameter("wk", [D, D], f32r, isOutput=False)
    wv_d = nc.declare_dram_parameter("wv", [D, D], f32r, isOutput=False)
    wo_d = nc.declare_dram_parameter("wo", [D, D], f32r, isOutput=False)
    bq_d = nc.declare_dram_parameter("bq", [D], f32, isOutput=False)
    bk_d = nc.declare_dram_parameter("bk", [D], f32, isOutput=False)
    bv_d = nc.declare_dram_parameter("bv", [D], f32, isOutput=False)
    bo_d = nc.declare_dram_parameter("bo", [D], f32, isOutput=False)
    # wr: [Wr | Wr[:, e]] so column 8 is this core's own-expert logit
    wr_d = nc.declare_dram_parameter("wr", [D, E + 1], f32, isOutput=False)
    br_d = nc.declare_dram_parameter("br", [E + 1], f32, isOutput=False)
    o_d = nc.declare_dram_parameter("o", [B, D, N], f32, isOutput=True)

    qT_r = qT_d.rearrange("b (ki p) t -> b p ki t", p=P)
    kT_r = kT_d.rearrange("b (ki p) t -> b p ki t", p=P)
    vT_r = vT_d.rearrange("b (ki p) t -> b p ki t", p=P)
    wq_r = wq_d.rearrange("(ki p) o -> p ki o", p=P)
    wk_r = wk_d.rearrange("(ki p) o -> p ki o", p=P)
    wv_r = wv_d.rearrange("(ki p) o -> p ki o", p=P)
    wo_r = wo_d.rearrange("(ki p) o -> p ki o", p=P)
    wr_r = wr_d.rearrange("(ki p) e -> p ki e", p=P)

    import concourse.bass as bass

    def pbcast(ap, nparts):
        # partition-stride-0 DMA source: replicate a [..] dram vector to
        # nparts partitions
        return bass.AP(tensor=ap.tensor, offset=ap.offset,
                       ap=[[0, nparts]] + list(ap.ap))

    with TileContext(nc) as tc:
        with tc.tile_pool(name="const", bufs=1) as cst, \
             tc.tile_pool(name="kvlong", bufs=1) as kvl, \
             tc.tile_pool(name="drp", bufs=4, space="DRAM") as drp, \
             tc.tile_pool(name="psp", bufs=1, space="PSUM") as psp:

            ident = cst.tile([P, P], f32, tag="ident")
            make_identity(nc, ident[:])
            ones32 = cst.tile([P, P], f32, tag="ones32")
            nc.vector.memset(ones32[:], 1.0)
            ones_r = cst.tile([P, P], f32r, tag="ones_r")
            nc.vector.tensor_copy(ones_r[:], ones32[:])

            wq_sb = cst.tile([P, KO, D], f32r, tag="wq")
            wo_sb = cst.tile([P, KO, D], f32r, tag="wo")
            for ki in range(KO):
                nc.sync.dma_start(wq_sb[:, ki], wq_r[:, ki])
                nc.sync.dma_start(wo_sb[:, ki], wo_r[:, ki])
            wr_sb = cst.tile([P, KO, E + 1], f32, tag="wr")
            nc.sync.dma_start(wr_sb[:], wr_r[:])

            bq_sb = cst.tile([P, KO], f32, tag="bq")
            bk_sb = cst.tile([P, KO], f32, tag="bk")
            bo_sb = cst.tile([P, KO], f32, tag="bo")
            nc.sync.dma_start(bq_sb[:], bq_d.rearrange("(ko p) -> p ko", p=P))
            nc.sync.dma_start(bk_sb[:], bk_d.rearrange("(ko p) -> p ko", p=P))
            nc.sync.dma_start(bo_sb[:], bo_d.rearrange("(ko p) -> p ko", p=P))
            bv_bc = cst.tile([P, D], f32, tag="bv")
            nc.gpsimd.dma_start(bv_bc[:], pbcast(bv_d[:], P))
            br_bc = cst.tile([P, E + 1], f32, tag="br")
            nc.gpsimd.dma_start(br_bc[:], pbcast(br_d[:], P))

            KT = kvl.tile([P, KO, MK], f32r, tag="KT")
            V = kvl.tile([P, MK // P, H * (HD + 1)], f32r, tag="V")

            import contextlib
            rep_ctx = (tc.For_i(0, KREP, 1) if KREP > 1
                       else contextlib.nullcontext())
            with rep_ctx:
              for b in range(B):
                  # ---- K/V setup for this batch ----
                  with tc.tile_pool(name="setup", bufs=1) as stp, \
                       tc.tile_pool(name="setups", bufs=2) as stps:
                      kT = stp.tile([P, KO, MK], f32r, tag="kT")
                      vT = stp.tile([P, KO, MK], f32r, tag="vT")
                      for ki in range(KO):
                          nc.sync.dma_start(kT[:, ki], kT_r[b, :, ki])
                          nc.sync.dma_start(vT[:, ki], vT_r[b, :, ki])
                      # KT = wk^T @ kT + bk   (dout on partitions, mk free)
                      for ko in range(KO):
                          wkt = stps.tile([P, KO, P], f32r, tag="wkt")
                          nc.sync.dma_start(wkt[:], wk_r[:, :, ko * P:(ko + 1) * P])
                          pk = psp.tile([P, MK], f32, tag="big", bufs=2)
                          for ki in range(KO):
                              nc.tensor.matmul(pk[:], wkt[:, ki], kT[:, ki],
                                               start=(ki == 0), stop=(ki == KO - 1))
                          nc.vector.tensor_scalar(
                              KT[:, ko], pk[:], bk_sb[:, ko:ko + 1], None, Op.add)
                      # V natural [mk, dout] = vT^T @ wv + bv, interleaved with a
                      # ones column every HD+1 so AV also produces the softmax sum
                      vview = V[:].rearrange("p m (h c) -> p m h c", c=HD + 1)
                      nc.vector.tensor_copy(
                          vview[:, :, :, HD],
                          ones32[:, :(MK // P) * H].rearrange(
                              "p (m h) -> p m h", m=MK // P))
                      for half in range(2):
                          wvt = stp.tile([P, KO, D // 2], f32r, tag="wvt")
                          nc.sync.dma_start(
                              wvt[:], wv_r[:, :, half * (D // 2):(half + 1) * (D // 2)])
                          for mt in range(MK // P):
                              pv = psp.tile([P, D // 2], f32, tag="big", bufs=2)
                              for ki in range(KO):
                                  nc.tensor.matmul(
                                      pv[:], vT[:, ki, mt * P:(mt + 1) * P],
                                      wvt[:, ki],
                                      start=(ki == 0), stop=(ki == KO - 1))
                              hsl = slice(half * (H // 2), (half + 1) * (H // 2))
                              nc.vector.tensor_tensor(
                                  vview[:, mt, hsl, :HD],
                                  pv[:].rearrange("p (h c) -> p h c", c=HD),
                                  bv_bc[:, half * (D // 2):(half + 1) * (D // 2)]
                                  .rearrange("p (h c) -> p h c", c=HD),
                                  Op.add)

                  # ---- chunk loop ----
                  with tc.tile_pool(name="chunk", bufs=1) as chk, \
                       tc.tile_pool(name="chks", bufs=2) as chs, \
                       tc.tile_pool(name="pt_pool", bufs=4) as ptp, \
                       tc.tile_pool(name="fin_pool", bufs=2) as fpl:
                      for c in range(NCH):
                          tok0 = c * NQC
                          qTc = chk.tile([P, KO, NQC], f32r, tag="qTc")
                          for ki in range(KO):
                              nc.sync.dma_start(
                                  qTc[:, ki], qT_r[b, :, ki, tok0:tok0 + NQC])

                          # ---- router ----
                          Lg = chs.tile([P, NQC // P, E + 1], f32, tag="Lg")
                          for tt in range(NQC // P):
                              pr = psp.tile([P, E + 1], f32, tag="big", bufs=2)
                              for ki in range(KO):
                                  nc.tensor.matmul(
                                      pr[:],
                                      qTc[:, ki, tt * P:(tt + 1) * P].bitcast(f32),
                                      wr_sb[:, ki],
                                      start=(ki == 0), stop=(ki == KO - 1))
                              nc.vector.tensor_tensor(Lg[:, tt], pr[:], br_bc[:],
                                                      Op.add)
                          m1 = chs.tile([P, NQC // P], f32, tag="m1")
                          m2 = chs.tile([P, NQC // P], f32, tag="m2")
                          msk = chs.tile([P, NQC // P, E], f32, tag="msk")
                          nc.vector.tensor_reduce(m1[:], Lg[:, :, :E],
                                                  mybir.AxisListType.X, Op.max)
                          nc.vector.tensor_tensor(
                              msk[:], Lg[:, :, :E],
                              m1[:, :, None].to_broadcast((P, NQC // P, E)),
                              Op.is_equal)
                          nc.vector.tensor_scalar(msk[:], msk[:], -1e30, None,
                                                  Op.mult)
                          nc.vector.tensor_tensor(msk[:], Lg[:, :, :E], msk[:],
                                                  Op.add)
                          nc.vector.tensor_reduce(m2[:], msk[:],
                                                  mybir.AxisListType.X, Op.max)
                          dd = chs.tile([P, NQC // P], f32, tag="dd")
                          w1 = chs.tile([P, NQC // P], f32, tag="w1")
                          nc.vector.tensor_tensor(dd[:], m2[:], m1[:], Op.subtract)
                          nc.scalar.activation(w1[:], dd[:], Af.Exp)
                          nc.vector.tensor_scalar(w1[:], w1[:], 1.0, None, Op.add)
                          with nc.allow_low_precision(reason="router sigmoid"):
                              nc.vector.reciprocal(w1[:], w1[:])
                          eq1 = chs.tile([P, NQC // P], f32, tag="eq1")
                          eq2 = chs.tile([P, NQC // P], f32, tag="eq2")
                          we = chs.tile([P, NQC // P], f32, tag="we")
                          nc.vector.tensor_tensor(eq1[:], Lg[:, :, E], m1[:],
                                                  Op.is_equal)
                          nc.vector.tensor_tensor(eq2[:], Lg[:, :, E], m2[:],
                                                  Op.is_equal)
                          nc.vector.tensor_tensor(eq1[:], eq1[:], w1[:], Op.mult)
                          # w2 = 1 - w1
                          nc.vector.tensor_scalar(w1[:], w1[:], -1.0, 1.0,
                                                  Op.mult, Op.add)
                          nc.vector.tensor_tensor(eq2[:], eq2[:], w1[:], Op.mult)
                          nc.vector.tensor_tensor(we[:], eq1[:], eq2[:], Op.add)
                          wrow = chs.tile([1, NQC], f32, tag="wrow", bufs=1)
                          for tt in range(NQC // P):
                              pw = psp.tile([1, P], f32, tag="big", bufs=2)
                              nc.tensor.transpose(pw[:], we[:, tt:tt + 1],
                                                  ident[:])
                              nc.vector.tensor_copy(
                                  wrow[0:1, tt * P:(tt + 1) * P], pw[0:1, :])
                          wdr = drp.tile([1, NQC], f32, tag="wdr")
                          nc.sync.dma_start(wdr[:], wrow[0:1, :])
                          w_sb = chs.tile([P, NQC], f32, tag="w_sb", bufs=1)
                          nc.gpsimd.dma_start(w_sb[:], pbcast(wdr[0, :], P))

                          # ---- Q projection (scale folded in) ----
                          Qc = chk.tile([P, KO, NQC], f32r, tag="Qc", bufs=2)
                          for ko in range(KO):
                              pq = psp.tile([P, NQC], f32, tag="big", bufs=2)
                              for ki in range(KO):
                                  nc.tensor.matmul(
                                      pq[:], wq_sb[:, ki, ko * P:(ko + 1) * P],
                                      qTc[:, ki],
                                      start=(ki == 0), stop=(ki == KO - 1))
                              nc.vector.tensor_scalar(
                                  Qc[:, ko], pq[:], bq_sb[:, ko:ko + 1], SCALE,
                                  Op.add, Op.mult)

                          # ---- heads ----
                          O_sb = chk.tile([P, KO, NQC], f32r, tag="O_sb", bufs=2)
                          for h in range(H):
                              p0 = (h % 2) * HD
                              koh = h // 2
                              po = psp.tile([HD + 1, NQC], f32, tag="po", bufs=2)
                              for pair in range(MK // P // 2):
                                  ps2 = psp.tile([P, 2, NQC], f32, tag="ps2",
                                                 bufs=2)
                                  for j in range(2):
                                      mt = pair * 2 + j
                                      nc.tensor.matmul(
                                          ps2[:, j],
                                          KT[p0:p0 + HD, koh,
                                             mt * P:(mt + 1) * P],
                                          Qc[p0:p0 + HD, koh],
                                          start=True, stop=True)
                                  pe2 = ptp.tile([P, 2, NQC], f32r, tag="pe", bufs=2)
                                  nc.scalar.activation(
                                      pe2[:], ps2[:],
                                      Af.Copy if "noexp" in ABL else Af.Exp)
                                  for j in range(2):
                                      mt = pair * 2 + j
                                      nc.tensor.matmul(
                                          po[:],
                                          V[:, mt,
                                            h * (HD + 1):(h + 1) * (HD + 1)],
                                          pe2[:, j],
                                          start=(mt == 0),
                                          stop=(mt == MK // P - 1))
                              if "nonorm" in ABL:
                                  nc.vector.tensor_copy(
                                      O_sb[p0:p0 + HD, koh], po[:HD, :])
                              else:
                                  recr = ptp.tile([1, NQC], f32r, tag="recr",
                                                  bufs=2)
                                  with nc.allow_low_precision(
                                          reason="softmax denom recip"):
                                      nc.vector.reciprocal(recr[0:1, :],
                                                           po[HD:HD + 1, :])
                                  p2 = psp.tile([HD, NQC], f32, tag="big",
                                                bufs=2)
                                  nc.tensor.matmul(p2[:], ones_r[0:1, :HD],
                                                   recr[0:1, :], start=True,
                                                   stop=True)
                                  rb = ptp.tile([HD, NQC], f32, tag="rb", bufs=2)
                                  nc.vector.tensor_copy(rb[:], p2[:])
                                  nc.vector.tensor_tensor(
                                      O_sb[p0:p0 + HD, koh], po[:HD, :], rb[:],
                                      Op.mult)

                          # ---- output projection + bias + router weight ----
                          for ko in range(KO):
                              pf = psp.tile([P, NQC], f32, tag="big", bufs=2)
                              for ki in range(KO):
                                  nc.tensor.matmul(
                                      pf[:], wo_sb[:, ki, ko * P:(ko + 1) * P],
                                      O_sb[:, ki],
                                      start=(ki == 0), stop=(ki == KO - 1))
                              fin = fpl.tile([P, NQC], f32, tag="fin")
                              nc.vector.tensor_scalar(
                                  fin[:], pf[:], bo_sb[:, ko:ko + 1], None, Op.add)
                              nc.vector.tensor_tensor(fin[:], fin[:], w_sb[:],
                                                      Op.mult)
                              nc.sync.dma_start(
                                  o_d[b, ko * P:(ko + 1) * P,
                                      tok0:tok0 + NQC], fin[:])
    nc.finalize()
    return nc


def _get_nc():
    global _NC
    if _NC is None:
        _NC = _build_nc()
    return _NC


def build_in_maps(inputs):
    ins = {k: np.asarray(v, dtype=np.float32) for k, v in inputs.items()}
    Wr = ins["Wr"]
    br = ins["br"]
    qT = np.ascontiguousarray(ins["queries"].transpose(0, 2, 1))
    in_maps = []
    for e in range(CORES):
        in_maps.append({
            "qT": qT,
            "kT": np.ascontiguousarray(
                ins["keys"][:, e * MK:(e + 1) * MK, :].transpose(0, 2, 1)),
            "vT": np.ascontiguousarray(
                ins["values"][:, e * MK:(e + 1) * MK, :].transpose(0, 2, 1)),
            "wq": ins["Wq"][e], "wk": ins["Wk"][e],
            "wv": ins["Wv"][e], "wo": ins["Wo"][e],
            "bq": ins["bq"][e], "bk": ins["bk"][e],
            "bv": ins["bv"][e], "bo": ins["bo"][e],
            "wr": np.ascontiguousarray(
                np.concatenate([Wr, Wr[:, e:e + 1]], axis=1)),
            "br": np.ascontiguousarray(
                np.concatenate([br, br[e:e + 1]], axis=0)),
        })
    return in_maps


def kernel(**inputs) -> np.ndarray:
    from concourse.bass_utils import run_bass_kernel_spmd

    in_maps = build_in_maps(inputs)
    nc = _get_nc()
    res = run_bass_kernel_spmd(nc, in_maps, list(range(CORES))).results
    acc = res[0]["o"].astype(np.float32)
    for e in range(1, CORES):
        acc = acc + res[e]["o"]
    return np.ascontiguousarray(acc.transpose(0, 2, 1))



# revision 3
# speedup vs baseline: 1.9804x; 1.9804x over previous
"""MixtureAttention (MoE attention routing) Trainium2 kernel.

Strategy: expert-parallel over 8 NeuronCores (one expert per core).
Each core computes its expert's full attention output for all tokens,
multiplies by the per-token router weight (0 for tokens that did not
select this expert in their top-2), and the host sums the 8 per-core
outputs — the top-k combine becomes a plain sum because the router
weight is zero for non-selected experts.

Router math on device (per token): top-2 of 8 logits == (m1, m2) the
two largest logits; softmax over them gives w1 = sigmoid(m1-m2),
w2 = 1-w1; this core's weight is w1/w2/0 by comparing its own logit
against m1/m2 (exact float equality; ties are measure-zero).  The
router matmul runs in exact fp32 so the top-2 selection matches the
reference; everything else runs in bf16 on the PE (fp32 PSUM
accumulate), which keeps the end-to-end max-rel error ~3e-3.

Layout: activations are kept transposed ("T-layout", feature on
partitions, token on free dim) so every matmul contracts along
partitions; q/k/v are pre-transposed on the host (the host also
supplies a bf16 copy of qT for the Q-projection; the fp32 copy feeds
the router).  Per (batch, 512-token chunk):
  Q = wq^T qT (+bq, *hd^-0.5) -> per head:
  S^T[mk,nq] = K_h^T(lhsT) @ Q_h -> exp on ACT (ONLY Exp — any other
  ACT function would thrash the activation table) -> AV with a ones
  column appended to V (65th column) so the softmax denominator lands
  in psum row 64 -> reciprocal_approx_fast (DVE) + partition_broadcast
  (Pool engine, keeps PE/DVE free) -> normalize -> O-proj ->
  (pf + bo) * w fused in one scalar_tensor_tensor pass -> out.

Both batches' K/V projections and router weights are computed in one
setup phase up front (router weight rows are transposed via the PE and
broadcast once per batch with partition_broadcast), so the 8 chunk
iterations run back-to-back with no mid-kernel phase change.
"""

import numpy as np

B, N, D, E, H = 2, 2048, 1024, 8, 16
MK = 512            # keys/values chunk per expert (M // E)
HD = D // H         # 64
P = 128
KO = D // P         # 8
NQC = 512           # token chunk (matmul free dim)
NCH = N // NQC      # 4
SCALE = HD ** -0.5
CORES = 8

_NC = None
import os
KREP = int(os.environ.get("KREP", "1"))


def _build_nc():
    import concourse.bacc as bacc
    import concourse.mybir as mybir
    from concourse.tile import TileContext
    from concourse.masks import make_identity

    f32 = mybir.dt.float32
    bf16 = mybir.dt.bfloat16
    Af = mybir.ActivationFunctionType
    Op = mybir.AluOpType

    nc = bacc.Bacc("TRN2", target_bir_lowering=False)

    qT_d = nc.declare_dram_parameter("qT", [B, D, N], f32, isOutput=False)
    qTb_d = nc.declare_dram_parameter("qTb", [B, D, N], bf16, isOutput=False)
    kT_d = nc.declare_dram_parameter("kT", [B, D, MK], bf16, isOutput=False)
    vT_d = nc.declare_dram_parameter("vT", [B, D, MK], bf16, isOutput=False)
    wq_d = nc.declare_dram_parameter("wq", [D, D], bf16, isOutput=False)
    wk_d = nc.declare_dram_parameter("wk", [D, D], bf16, isOutput=False)
    wv_d = nc.declare_dram_parameter("wv", [D, D], bf16, isOutput=False)
    wo_d = nc.declare_dram_parameter("wo", [D, D], bf16, isOutput=False)
    bq_d = nc.declare_dram_parameter("bq", [D], f32, isOutput=False)
    bk_d = nc.declare_dram_parameter("bk", [D], f32, isOutput=False)
    bv_d = nc.declare_dram_parameter("bv", [D], f32, isOutput=False)
    bo_d = nc.declare_dram_parameter("bo", [D], f32, isOutput=False)
    # wr: [Wr | Wr[:, e]] so column 8 is this core's own-expert logit
    wr_d = nc.declare_dram_parameter("wr", [D, E + 1], f32, isOutput=False)
    br_d = nc.declare_dram_parameter("br", [E + 1], f32, isOutput=False)
    o_d = nc.declare_dram_parameter("o", [B, D, N], f32, isOutput=True)

    qT_r = qT_d.rearrange("b (ki p) t -> b p ki t", p=P)
    qTb_r = qTb_d.rearrange("b (ki p) t -> b p ki t", p=P)
    kT_r = kT_d.rearrange("b (ki p) t -> b p ki t", p=P)
    vT_r = vT_d.rearrange("b (ki p) t -> b p ki t", p=P)
    wq_r = wq_d.rearrange("(ki p) o -> p ki o", p=P)
    wk_r = wk_d.rearrange("(ki p) o -> p ki o", p=P)
    wv_r = wv_d.rearrange("(ki p) o -> p ki o", p=P)
    wo_r = wo_d.rearrange("(ki p) o -> p ki o", p=P)
    wr_r = wr_d.rearrange("(ki p) e -> p ki e", p=P)

    NT = N // P         # 16 token tiles per batch (router)

    with TileContext(nc) as tc:
        with tc.tile_pool(name="const", bufs=1) as cst, \
             tc.tile_pool(name="kvlong", bufs=1) as kvl, \
             tc.tile_pool(name="psp", bufs=1, space="PSUM") as psp:

            ident = cst.tile([P, P], f32, tag="ident")
            make_identity(nc, ident[:])

            wq_sb = cst.tile([P, KO, D], bf16, tag="wq")
            wk_sb = cst.tile([P, KO, D], bf16, tag="wk")
            wv_sb = cst.tile([P, KO, D], bf16, tag="wv")
            wo_sb = cst.tile([P, KO, D], bf16, tag="wo")
            nc.sync.dma_start(wq_sb[:], wq_r[:])
            nc.sync.dma_start(wk_sb[:], wk_r[:])
            nc.sync.dma_start(wv_sb[:], wv_r[:])
            nc.sync.dma_start(wo_sb[:], wo_r[:])
            wr_sb = cst.tile([P, KO, E + 1], f32, tag="wr")
            nc.sync.dma_start(wr_sb[:], wr_r[:])

            bq_sb = cst.tile([P, KO], f32, tag="bq")
            bk_sb = cst.tile([P, KO], f32, tag="bk")
            bo_sb = cst.tile([P, KO], f32, tag="bo")
            nc.sync.dma_start(bq_sb[:], bq_d.rearrange("(ko p) -> p ko", p=P))
            nc.sync.dma_start(bk_sb[:], bk_d.rearrange("(ko p) -> p ko", p=P))
            nc.sync.dma_start(bo_sb[:], bo_d.rearrange("(ko p) -> p ko", p=P))
            bv_row = cst.tile([1, D], f32, tag="bv_row")
            nc.sync.dma_start(bv_row[:], bv_d[None, :])
            bv_bc = cst.tile([P, D], f32, tag="bv")
            nc.gpsimd.partition_broadcast(bv_bc[:], bv_row[0:1, :], channels=P)
            br_row = cst.tile([1, E + 1], f32, tag="br_row")
            nc.sync.dma_start(br_row[:], br_d[None, :])
            br_bc = cst.tile([P, E + 1], f32, tag="br")
            nc.gpsimd.partition_broadcast(br_bc[:], br_row[0:1, :], channels=P)
            ones_bf = cst.tile([P, (MK // P) * H], bf16, tag="ones_bf")
            nc.vector.memset(ones_bf[:], 1.0)

            import contextlib
            rep_ctx = (tc.For_i(0, KREP, 1) if KREP > 1
                       else contextlib.nullcontext())
            with rep_ctx:
              KT = kvl.tile([P, B, KO, MK], bf16, tag="KT", bufs=1)
              V = kvl.tile([P, B, MK // P, H * (HD + 1)], bf16, tag="V",
                           bufs=1)
              w_all = kvl.tile([P, B, N], f32, tag="w_all", bufs=1)

              # ---- setup phase: K/V projections + router, both batches ----
              with tc.tile_pool(name="setup", bufs=2) as stp, \
                   tc.tile_pool(name="rsmall", bufs=2) as rtp:
                for b in range(B):
                    kraw = stp.tile([P, KO, MK], bf16, tag="kraw")
                    vraw = stp.tile([P, KO, MK], bf16, tag="vraw")
                    nc.sync.dma_start(kraw[:], kT_r[b])
                    nc.sync.dma_start(vraw[:], vT_r[b])
                    # KT = wk^T @ kT + bk   (dout on partitions, mk free)
                    for ko in range(KO):
                        pk = psp.tile([P, NQC], f32, tag="big", bufs=2)
                        for ki in range(KO):
                            nc.tensor.matmul(
                                pk[:], wk_sb[:, ki, ko * P:(ko + 1) * P],
                                kraw[:, ki],
                                start=(ki == 0), stop=(ki == KO - 1))
                        nc.vector.tensor_scalar(
                            KT[:, b, ko], pk[:], bk_sb[:, ko:ko + 1], None,
                            Op.add)
                    # V natural [mk, dout] = vT^T @ wv + bv, interleaved with
                    # a ones column every HD+1 so AV also produces the
                    # softmax denominator
                    vview = V[:, b].rearrange("p m (h c) -> p m h c", c=HD + 1)
                    nc.vector.tensor_copy(
                        vview[:, :, :, HD],
                        ones_bf[:].rearrange("p (m h) -> p m h", m=MK // P))
                    for half in range(2):
                        for mt in range(MK // P):
                            pv = psp.tile([P, NQC], f32, tag="big", bufs=2)
                            for ki in range(KO):
                                nc.tensor.matmul(
                                    pv[:], vraw[:, ki, mt * P:(mt + 1) * P],
                                    wv_sb[:, ki,
                                          half * (D // 2):(half + 1) * (D // 2)],
                                    start=(ki == 0), stop=(ki == KO - 1))
                            hsl = slice(half * (H // 2), (half + 1) * (H // 2))
                            nc.vector.tensor_tensor(
                                vview[:, mt, hsl, :HD],
                                pv[:].rearrange("p (h c) -> p h c", c=HD),
                                bv_bc[:, half * (D // 2):(half + 1) * (D // 2)]
                                .rearrange("p (h c) -> p h c", c=HD),
                                Op.add)

                    # ---- router for this batch (exact fp32) ----
                    Lg = rtp.tile([P, NT, E + 1], f32, tag="Lg")
                    for s4 in range(N // NQC):
                        qTr = stp.tile([P, KO, NQC], f32, tag="qTr")
                        nc.sync.dma_start(
                            qTr[:], qT_r[b, :, :, s4 * NQC:(s4 + 1) * NQC])
                        for tt in range(NQC // P):
                            pr = psp.tile([P, NQC], f32, tag="big", bufs=2)
                            for ki in range(KO):
                                nc.tensor.matmul(
                                    pr[:, :E + 1],
                                    qTr[:, ki, tt * P:(tt + 1) * P],
                                    wr_sb[:, ki],
                                    start=(ki == 0), stop=(ki == KO - 1))
                            nc.vector.tensor_tensor(
                                Lg[:, s4 * (NQC // P) + tt], pr[:, :E + 1],
                                br_bc[:], Op.add)
                    m1 = rtp.tile([P, NT], f32, tag="m1")
                    m2 = rtp.tile([P, NT], f32, tag="m2")
                    msk = rtp.tile([P, NT, E], f32, tag="msk")
                    nc.vector.tensor_reduce(m1[:], Lg[:, :, :E],
                                            mybir.AxisListType.X, Op.max)
                    nc.vector.tensor_tensor(
                        msk[:], Lg[:, :, :E],
                        m1[:, :, None].to_broadcast((P, NT, E)),
                        Op.is_equal)
                    nc.vector.tensor_scalar(msk[:], msk[:], -1e30, None,
                                            Op.mult)
                    nc.vector.tensor_tensor(msk[:], Lg[:, :, :E], msk[:],
                                            Op.add)
                    nc.vector.tensor_reduce(m2[:], msk[:],
                                            mybir.AxisListType.X, Op.max)
                    dd = rtp.tile([P, NT], f32, tag="dd")
                    w1 = rtp.tile([P, NT], f32, tag="w1")
                    nc.vector.tensor_tensor(dd[:], m2[:], m1[:], Op.subtract)
                    nc.scalar.activation(w1[:], dd[:], Af.Exp)
                    nc.vector.tensor_scalar(w1[:], w1[:], 1.0, None, Op.add)
                    with nc.allow_low_precision(reason="router sigmoid"):
                        nc.vector.reciprocal(w1[:], w1[:])
                    eq1 = rtp.tile([P, NT], f32, tag="eq1")
                    eq2 = rtp.tile([P, NT], f32, tag="eq2")
                    we = rtp.tile([P, NT], f32, tag="we")
                    nc.vector.tensor_tensor(eq1[:], Lg[:, :, E], m1[:],
                                            Op.is_equal)
                    nc.vector.tensor_tensor(eq2[:], Lg[:, :, E], m2[:],
                                            Op.is_equal)
                    nc.vector.tensor_tensor(eq1[:], eq1[:], w1[:], Op.mult)
                    # w2 = 1 - w1
                    nc.vector.tensor_scalar(w1[:], w1[:], -1.0, 1.0,
                                            Op.mult, Op.add)
                    nc.vector.tensor_tensor(eq2[:], eq2[:], w1[:], Op.mult)
                    nc.vector.tensor_tensor(we[:], eq1[:], eq2[:], Op.add)
                    wrow = rtp.tile([1, N], f32, tag="wrow")
                    for tt in range(NT):
                        pw = psp.tile([P, NQC], f32, tag="big", bufs=2)
                        nc.tensor.transpose(pw[0:1, :P], we[:, tt:tt + 1],
                                            ident[:])
                        nc.vector.tensor_copy(
                            wrow[0:1, tt * P:(tt + 1) * P], pw[0:1, :P])
                    nc.gpsimd.partition_broadcast(w_all[:, b], wrow[0:1, :],
                                                  channels=P)

              # ---- chunk loop ----
              with tc.tile_pool(name="chunk", bufs=2) as chk, \
                   tc.tile_pool(name="pt_pool", bufs=4) as ptp, \
                   tc.tile_pool(name="fin_pool", bufs=2) as fpl:
                for b in range(B):
                    for c in range(NCH):
                        tok0 = c * NQC
                        qTcb = chk.tile([P, KO, NQC], bf16, tag="qTcb")
                        nc.sync.dma_start(
                            qTcb[:], qTb_r[b, :, :, tok0:tok0 + NQC])

                        # ---- Q projection (scale folded in) ----
                        Qc = chk.tile([P, KO, NQC], bf16, tag="Qc")
                        for ko in range(KO):
                            pq = psp.tile([P, NQC], f32, tag="big", bufs=2)
                            for ki in range(KO):
                                nc.tensor.matmul(
                                    pq[:], wq_sb[:, ki, ko * P:(ko + 1) * P],
                                    qTcb[:, ki],
                                    start=(ki == 0), stop=(ki == KO - 1))
                            nc.vector.tensor_scalar(
                                Qc[:, ko], pq[:], bq_sb[:, ko:ko + 1], SCALE,
                                Op.add, Op.mult)

                        # ---- heads ----
                        O_sb = chk.tile([P, KO, NQC], bf16, tag="O_sb")
                        for h in range(H):
                            p0 = (h % 2) * HD
                            koh = h // 2
                            po = psp.tile([HD + 1, NQC], f32, tag="po",
                                          bufs=2)
                            for pair in range(MK // P // 2):
                                ps2 = psp.tile([P, 2, NQC], f32, tag="ps2",
                                               bufs=2)
                                for j in range(2):
                                    mt = pair * 2 + j
                                    nc.tensor.matmul(
                                        ps2[:, j],
                                        KT[p0:p0 + HD, b, koh,
                                           mt * P:(mt + 1) * P],
                                        Qc[p0:p0 + HD, koh],
                                        start=True, stop=True)
                                pe2 = ptp.tile([P, 2, NQC], bf16, tag="pe",
                                               bufs=4)
                                nc.scalar.activation(pe2[:], ps2[:], Af.Exp)
                                for j in range(2):
                                    mt = pair * 2 + j
                                    nc.tensor.matmul(
                                        po[:],
                                        V[:, b, mt,
                                          h * (HD + 1):(h + 1) * (HD + 1)],
                                        pe2[:, j],
                                        start=(mt == 0),
                                        stop=(mt == MK // P - 1))
                            recr = ptp.tile([1, NQC], f32, tag="recr",
                                            bufs=2)
                            with nc.allow_low_precision(
                                    reason="softmax denom recip"):
                                nc.vector.reciprocal(
                                    recr[0:1, :], po[HD:HD + 1, :])
                            rb = ptp.tile([HD, NQC], f32, tag="rb", bufs=3)
                            nc.gpsimd.partition_broadcast(
                                rb[:], recr[0:1, :], channels=HD)
                            nc.vector.tensor_tensor(
                                O_sb[p0:p0 + HD, koh], po[:HD, :], rb[:],
                                Op.mult)

                        # ---- output projection: (pf + bo) * w, fused ----
                        for ko in range(KO):
                            pf = psp.tile([P, NQC], f32, tag="big", bufs=2)
                            for ki in range(KO):
                                nc.tensor.matmul(
                                    pf[:], wo_sb[:, ki, ko * P:(ko + 1) * P],
                                    O_sb[:, ki],
                                    start=(ki == 0), stop=(ki == KO - 1))
                            fin = fpl.tile([P, NQC], f32, tag="fin")
                            nc.vector.scalar_tensor_tensor(
                                fin[:], pf[:], bo_sb[:, ko:ko + 1],
                                w_all[:, b, tok0:tok0 + NQC],
                                Op.add, Op.mult)
                            nc.sync.dma_start(
                                o_d[b, ko * P:(ko + 1) * P,
                                    tok0:tok0 + NQC], fin[:])
    nc.finalize()
    return nc


def _get_nc():
    global _NC
    if _NC is None:
        _NC = _build_nc()
    return _NC


def build_in_maps(inputs):
    import ml_dtypes
    bf16 = ml_dtypes.bfloat16
    ins = {k: np.asarray(v, dtype=np.float32) for k, v in inputs.items()}
    Wr = ins["Wr"]
    br = ins["br"]
    qT = np.ascontiguousarray(ins["queries"].transpose(0, 2, 1))
    qTb = qT.astype(bf16)
    in_maps = []
    for e in range(CORES):
        in_maps.append({
            "qT": qT,
            "qTb": qTb,
            "kT": np.ascontiguousarray(
                ins["keys"][:, e * MK:(e + 1) * MK, :].transpose(0, 2, 1)
            ).astype(bf16),
            "vT": np.ascontiguousarray(
                ins["values"][:, e * MK:(e + 1) * MK, :].transpose(0, 2, 1)
            ).astype(bf16),
            "wq": ins["Wq"][e].astype(bf16), "wk": ins["Wk"][e].astype(bf16),
            "wv": ins["Wv"][e].astype(bf16), "wo": ins["Wo"][e].astype(bf16),
            "bq": ins["bq"][e], "bk": ins["bk"][e],
            "bv": ins["bv"][e], "bo": ins["bo"][e],
            "wr": np.ascontiguousarray(
                np.concatenate([Wr, Wr[:, e:e + 1]], axis=1)),
            "br": np.ascontiguousarray(
                np.concatenate([br, br[e:e + 1]], axis=0)),
        })
    return in_maps


def kernel(**inputs) -> np.ndarray:
    from concourse.bass_utils import run_bass_kernel_spmd

    in_maps = build_in_maps(inputs)
    nc = _get_nc()
    res = run_bass_kernel_spmd(nc, in_maps, list(range(CORES))).results
    acc = res[0]["o"].astype(np.float32)
    for e in range(1, CORES):
        acc = acc + res[e]["o"]
    return np.ascontiguousarray(acc.transpose(0, 2, 1))
